# revision 1
# baseline (speedup 1.0000x reference)
"""Trainium2 Bass kernel for nn_BatchedQNodeLayer (8-qubit batched QNode).

Math: for an RX-angle-embedded product state pushed through a fixed
(theta-dependent) 2-layer strongly-entangling circuit and measured with
<Z_0>, the output is

    out_b = 0.5 + 0.5 * <psi(x_b)| M(theta) |psi(x_b)>

M expanded in the {I,Y,Z}^8 Pauli basis (X terms vanish for RX product
states) gives out_b as a multilinear form in per-wire features
[1, -sin(x_w), cos(x_w)].  The coefficient tensor factors hierarchically
(operator-Schmidt ranks are tiny for a shallow circuit; K=R1=R2=2 here),
and after pruning coefficients below 1e-5 (validated on the host against
the unpruned factors) the per-element device work is ~45 elementwise
MACs over sin/cos planes.  All coefficients are computed on the host
from theta (O(1) in batch) and baked into the instruction stream as
immediates; constant sub-chains are folded algebraically.

Layout per core: batch shard of 16384 elements as [128 partitions, 128
free] planes; sin/cos via the ACT engine (range-reduced to [-pi, pi]
with the fp32 magic-rounding trick since the Sin table is only accurate
there); pair products via wide multi-dim-AP ops and MAC chains via
scalar_tensor_tensor on the vector engine; input via one big SWDGE
(gpsimd) DMA.  Measured ~33.5 us on-device per 8-core SPMD dispatch,
rel err ~7e-6 vs the fp64 reference.
"""

import sys

sys.path.insert(0, "/opt/trn_rl_repo")

import numpy as np

N_QUBITS = 8
DIM = 256
N_CORES = 8
B_TOTAL = 131072
B_CORE = B_TOTAL // N_CORES  # 16384
P = 128                      # partitions
J = B_CORE // P              # 128 free elems per partition

TWO_PI = float(2.0 * np.pi)
INV_2PI = float(1.0 / (2.0 * np.pi))
MAGIC = float(1.5 * 2**23)   # fp32 round-to-nearest-integer bias
HALF_PI = float(np.pi / 2.0)


# ----------------------------------------------------------------------------
# Host-side precompute: theta -> hierarchical factor tensors
# ----------------------------------------------------------------------------

def _evolved_observable(theta):
    """M = U^dag Z0 U as dense 256x256 complex128 (numpy only)."""
    def rot(phi, th, om):
        c, s = np.cos(th / 2), np.sin(th / 2)
        return np.array([
            [np.exp(-0.5j * (phi + om)) * c, -np.exp(0.5j * (phi - om)) * s],
            [np.exp(-0.5j * (phi - om)) * s, np.exp(0.5j * (phi + om)) * c]])

    U = np.eye(DIM, dtype=np.complex128)

    def apply_1q(U, g, w):
        Ur = U.reshape([2] * N_QUBITS + [DIM])
        Ur = np.moveaxis(Ur, w, 0)
        Ur = np.tensordot(g, Ur, axes=([1], [0]))
        Ur = np.moveaxis(Ur, 0, w)
        return Ur.reshape(DIM, DIM)

    def apply_cnot(U, c, t):
        rows = np.arange(DIM)
        cbit = (rows >> (N_QUBITS - 1 - c)) & 1
        perm = np.where(cbit == 1, rows ^ (1 << (N_QUBITS - 1 - t)), rows)
        return U[perm, :]

    for l in range(2):
        for w in range(N_QUBITS):
            U = apply_1q(U, rot(*theta[l, w]), w)
        r = (l % (N_QUBITS - 1)) + 1
        for w in range(N_QUBITS):
            U = apply_cnot(U, w, (w + r) % N_QUBITS)
    z0 = 1.0 - 2.0 * ((np.arange(DIM) >> (N_QUBITS - 1)) & 1)
    return U.conj().T @ (z0[:, None] * U)


def _iyz_tensor(M):
    """Pauli coefficients over {I,Y,Z}^8 (axis order I,Y,Z per wire)."""
    I2 = np.eye(2, dtype=np.complex128)
    X = np.array([[0, 1], [1, 0]], dtype=np.complex128)
    Y = np.array([[0, -1j], [1j, 0]], dtype=np.complex128)
    Z = np.array([[1, 0], [0, -1]], dtype=np.complex128)
    T = M.reshape([2] * 16)
    perm = []
    for w in range(N_QUBITS):
        perm += [w, 8 + w]
    T = np.transpose(T, perm).reshape([4] * N_QUBITS)
    A = np.zeros((4, 4), dtype=np.complex128)
    for p, Pm in enumerate([I2, X, Y, Z]):
        A[p] = (Pm.T / 2).reshape(-1)
    for w in range(N_QUBITS):
        T = np.moveaxis(np.tensordot(A, T, axes=([1], [w])), 0, w)
    C = T.real
    idx = [0, 2, 3]
    return C[np.ix_(idx, idx, idx, idx, idx, idx, idx, idx)].copy()


def _factorize(theta, tol=1e-9):
    M = _evolved_observable(np.asarray(theta, np.float64))
    C = _iyz_tensor(M) * 0.5  # folds out = 0.5 + 0.5*ev
    S = C.reshape(81, 81)
    U, s, Vt = np.linalg.svd(S)
    K = max(1, int((s > s[0] * tol).sum()))
    A = U[:, :K] * np.sqrt(s[:K])
    Bv = Vt[:K].T * np.sqrt(s[:K])
    AL = A.reshape(9, 9, K)
    M1 = AL.reshape(9, 9 * K)
    P1, t1, Q1t = np.linalg.svd(M1, full_matrices=False)
    R1 = max(1, int((t1 > t1[0] * tol).sum()))
    W01 = P1[:, :R1] * np.sqrt(t1[:R1])                                  # [9,R1]
    V23 = Q1t[:R1].reshape(R1, 9, K) * np.sqrt(t1[:R1])[:, None, None]   # [R1,9,K]
    BR = Bv.reshape(9, 9, K).transpose(1, 0, 2)
    M2 = BR.reshape(9, 9 * K)
    P2, t2, Q2t = np.linalg.svd(M2, full_matrices=False)
    R2 = max(1, int((t2 > t2[0] * tol).sum()))
    W67 = P2[:, :R2] * np.sqrt(t2[:R2])                                  # [9,R2]
    V45 = Q2t[:R2].reshape(R2, 9, K) * np.sqrt(t2[:R2])[:, None, None]   # [R2,9,K]
    return dict(K=K, R1=R1, R2=R2, W01=W01, V23=V23, W67=W67, V45=V45)


def _prune_err(F, thr):
    """Max deviation of thr-pruned factors vs full, on random inputs."""
    rng = np.random.default_rng(0)
    x = rng.standard_normal((4096, N_QUBITS))
    sin, cos = np.sin(x), np.cos(x)

    def feats(wA, wB):
        SA, CA = sin[:, wA], cos[:, wA]
        SB, CB = sin[:, wB], cos[:, wB]
        one = np.ones_like(SA)
        return np.stack([one, -SB, CB, -SA, SA * SB, -SA * CB,
                         CA, -CA * SB, CA * CB], 1)

    f01, f23 = feats(0, 1), feats(2, 3)
    f45, f67 = feats(4, 5), feats(6, 7)

    def ev(W01, V23, W67, V45):
        u01 = f01 @ W01
        v23 = np.einsum('ba,mak->bmk', f23, V23)
        u67 = f67 @ W67
        v45 = np.einsum('bc,mck->bmk', f45, V45)
        uLk = np.einsum('bm,bmk->bk', u01, v23)
        uRk = np.einsum('bm,bmk->bk', u67, v45)
        return (uLk * uRk).sum(1)

    full = ev(F["W01"], F["V23"], F["W67"], F["V45"])
    pr = [np.where(np.abs(F[k]) > thr, F[k], 0.0)
          for k in ("W01", "V23", "W67", "V45")]
    return float(np.abs(full - ev(*pr)).max())


def _pick_prune_thr(F):
    for thr in (1e-5, 1e-6, 1e-7, 0.0):
        if _prune_err(F, thr) < 3e-5:
            return thr
    return 0.0


# ----------------------------------------------------------------------------
# Bass program
# ----------------------------------------------------------------------------

def _build_program(F, prune_thr=1e-5, safe_range=True):
    from concourse import bass, mybir, tile
    from concourse.vector_clock import ScopedClock

    class SafeTileContext(tile.TileContext):
        """This walrus rejects instructions carrying more than one sync
        wait.  After scheduling, park every extra wait on a same-engine
        nop inserted immediately before the instruction."""

        def schedule_and_allocate(self):
            ret = super().schedule_and_allocate()
            nc = self.nc
            for bb in list(nc.main_func.blocks):
                i = 0
                while i < len(bb.instructions):
                    ins = bb.instructions[i]
                    si = ins.sync_info
                    waits = list(si.on_wait or []) if si else []
                    lim = 1
                    if len(waits) > lim:
                        ins.sync_info = mybir.SyncInfo(
                            on_wait=waits[:lim], on_update=si.on_update)
                        rest = waits[lim:]
                        nops = []
                        while rest:
                            n = nc.engines[ins.engine].nop()
                            n.ins.sync_info = mybir.SyncInfo(
                                on_wait=rest[:1], on_update=[])
                            rest = rest[1:]
                            nops.append(n.ins)
                        for n in nops:
                            for blk in nc.main_func.blocks:
                                if n in blk.instructions:
                                    blk.instructions.remove(n)
                                    break
                        bb.instructions[i:i] = nops
                        i += len(nops)
                    i += 1
            return ret

    f32 = mybir.dt.float32
    OP = mybir.AluOpType
    AF = mybir.ActivationFunctionType

    nc = bass.Bass()
    x_in = nc.dram_tensor("x", [B_CORE, N_QUBITS], f32, kind="ExternalInput")
    y_out = nc.dram_tensor("out", [B_CORE, 1], f32, kind="ExternalOutput")

    with SafeTileContext(nc) as tc:
        with tc.tile_pool(name="pool", bufs=1) as pool:
            X = pool.tile([P, J * N_QUBITS], f32)        # (p, j*8+w)
            T1 = pool.tile([P, J * N_QUBITS], f32)
            Y = pool.tile([P, N_QUBITS * J], f32)        # w-major (p, w*128+j)
            # TRIG = [ sin block (w-major 1024) | cos block (1024) ]
            TRIG = pool.tile([P, 2 * N_QUBITS * J], f32)
            AB = pool.tile([P, N_QUBITS * J], f32)
            # PROD cols = (pair, a, b, j): a/b = 0:sin,1:cos of wA/wB
            PROD = pool.tile([P, 16 * J], f32)
            hp = pool.tile([P, 1], f32)

            # Preload the ACT Sin table before anything else on Scalar:
            # a tiny dummy activation with no data dependencies.
            warm = pool.tile([P, 1], f32)
            nc.scalar.activation(warm[:, :], warm[:, :], AF.Sin)

            nc.vector.memset(hp[:, :], HALF_PI)

            # input DMA: one big SWDGE transfer (gpsimd) — it spreads the
            # descriptors across queues internally and completes with a
            # single semaphore, beating chunked engine-direct DMAs
            xv = x_in.rearrange("(p j) w -> p (j w)", p=P)
            nc.gpsimd.dma_start(X[0:64, :], xv[0:64, :])
            nc.gpsimd.dma_start(X[64:128, :], xv[64:128, :])

            SIN = TRIG[:, 0:N_QUBITS * J]
            COS = TRIG[:, N_QUBITS * J:2 * N_QUBITS * J]
            H = 4 * J
            if safe_range:
                # |x| < 2pi guaranteed: half-angle path, no range reduction.
                # s2 = sin(x/2), c2 = cos(x/2) = sin(pi/2 - |x/2|), then
                # sin(x) = 2*s2*c2, cos(x) = 1 - 2*s2^2.  ACT reads X
                # strided and writes w-major directly.
                S2 = T1   # reuse
                C2 = Y    # reuse
                for h in range(2):
                    sl = slice(h * H, (h + 1) * H)
                    Xh = X[:, :].rearrange(
                        "p (j w) -> p w j", w=N_QUBITS)[:, 4 * h:4 * h + 4, :]
                    S2h = S2[:, sl].rearrange("p (w j) -> p w j", w=4)
                    ABh = AB[:, sl].rearrange("p (w j) -> p w j", w=4)
                    nc.scalar.activation(S2h, Xh, AF.Sin, scale=0.5)
                    nc.scalar.activation(ABh, Xh, AF.Abs, scale=0.5)
                    nc.scalar.activation(C2[:, sl], AB[:, sl], AF.Sin,
                                         bias=hp[:, :], scale=-1.0)
                    # sin(x) = (s2*2)*c2 ; cos(x) = (s2*-2)*s2 + 1
                    nc.vector.scalar_tensor_tensor(
                        SIN[:, sl], S2[:, sl], 2.0, C2[:, sl],
                        OP.mult, OP.mult)
                    nc.vector.scalar_tensor_tensor(
                        COS[:, sl], S2[:, sl], -2.0, S2[:, sl],
                        OP.mult, OP.mult)
                    nc.vector.tensor_scalar(COS[:, sl], COS[:, sl], 1.0, 1.0,
                                            OP.mult, OP.add)
            else:
                # range reduction: y = x - 2pi*round(x/(2pi)), w-major
                nc.vector.tensor_scalar(T1[:, :], X[:, :], INV_2PI, MAGIC,
                                        OP.mult, OP.add)
                nc.vector.tensor_scalar(T1[:, :], T1[:, :], MAGIC, None,
                                        OP.subtract)
                for w in range(N_QUBITS):
                    Yw = Y[:, w * J:(w + 1) * J]
                    T1w = T1[:, :].rearrange("p (j w) -> p w j",
                                             w=N_QUBITS)[:, w, :]
                    Xw = X[:, :].rearrange("p (j w) -> p w j",
                                           w=N_QUBITS)[:, w, :]
                    nc.vector.scalar_tensor_tensor(Yw, T1w, -TWO_PI, Xw,
                                                   OP.mult, OP.add)
                for h in range(2):
                    sl = slice(h * H, (h + 1) * H)
                    nc.scalar.activation(SIN[:, sl], Y[:, sl], AF.Sin)
                    nc.scalar.activation(AB[:, sl], Y[:, sl], AF.Abs)
                    nc.scalar.activation(COS[:, sl], AB[:, sl], AF.Sin,
                                         bias=hp[:, :], scale=-1.0)

            def Sw(w):
                return TRIG[:, w * J:(w + 1) * J]

            def Cw(w):
                return TRIG[:, (N_QUBITS + w) * J:(N_QUBITS + w + 1) * J]

            # all 16 pair products in four wide-AP ops (3 free dims max,
            # split by half so they chase the trig halves):
            # PROD[p, pr, a, b, j] = TRIG[p, a, 2pr, j] * TRIG[p, b, 2pr+1, j]
            tv = TRIG[:, :].rearrange("p (a pr t j) -> p a pr t j",
                                      a=2, pr=4, t=2)
            ov = PROD[:, :].rearrange("p (pr a b j) -> p pr a b j",
                                      pr=4, a=2, b=2)
            in2 = tv[:, :, :, 1:2, :].transpose([0, 2, 1, 3, 4]) \
                .squeeze(3)                     # [p, pr, b, j], b-stride 1024
            for h in range(2):
                pr = slice(2 * h, 2 * h + 2)
                for a in range(2):
                    in1 = tv[:, a:a + 1, pr, 0:1, :].squeeze(1) \
                        .broadcast_to([P, 2, 2, J])  # [p, pr, b(bcast), j]
                    out_a = ov[:, pr, a:a + 1, :, :].squeeze(2)
                    nc.vector.tensor_tensor(out_a, in1[:, :, :, :],
                                            in2[:, pr, :, :], OP.mult)

            def prod(pair_idx, a, b):
                base = (pair_idx * 4 + a * 2 + b) * J
                return PROD[:, base:base + J]

            PAIR_IDX = {(0, 1): 0, (2, 3): 1, (4, 5): 2, (6, 7): 3}
            PRUNE = float(prune_thr)

            def emit_chain(name, pair, w9):
                """q = sum_a w9[a]*mono_a over pair.  Returns None (zero),
                float (constant) or a tile.  mono a = 3*iA+iB, features
                [1, -s, c] per wire."""
                wA, wB = pair
                pi = PAIR_IDX[pair]
                cand = [
                    (Sw(wB), -w9[1]), (Cw(wB), w9[2]),
                    (Sw(wA), -w9[3]), (Cw(wA), w9[6]),
                    (prod(pi, 0, 0), w9[4]), (prod(pi, 0, 1), -w9[5]),
                    (prod(pi, 1, 0), -w9[7]), (prod(pi, 1, 1), w9[8]),
                ]
                terms = [(ap, c) for (ap, c) in cand if abs(c) > PRUNE]
                if not terms:
                    if abs(w9[0]) <= PRUNE:
                        return None
                    return float(w9[0])
                q = pool.tile([P, J], f32, tag=name)
                ap0, c0 = terms[0]
                nc.vector.tensor_scalar(q[:, :], ap0, float(c0), float(w9[0]),
                                        OP.mult, OP.add)
                for (ap, c) in terms[1:]:
                    nc.vector.scalar_tensor_tensor(q[:, :], ap, float(c),
                                                   q[:, :], OP.mult, OP.add)
                return q

            def emit_side(Wu, Vv, upair, vpair, tag):
                """Returns per-k (acc_tile_or_None, bias) for
                uX_k = sum_m chain(Wu[:,m]) * chain(Vv[m,:,k])."""
                R = Wu.shape[1]
                K = Vv.shape[2]
                us = [emit_chain(f"u{tag}{m}", upair, Wu[:, m])
                      for m in range(R)]
                outs = []
                for k in range(K):
                    merged = np.zeros(9)
                    mpairs = []
                    for m in range(R):
                        vcoef = Vv[m, :, k]
                        if not np.any(np.abs(vcoef) > PRUNE):
                            continue
                        if us[m] is None:
                            continue
                        if isinstance(us[m], float):
                            merged = merged + us[m] * vcoef
                        else:
                            mpairs.append((us[m], vcoef))
                    acc = None
                    bias = 0.0
                    if np.any(np.abs(merged) > PRUNE):
                        mc = emit_chain(f"w{tag}{k}", vpair, merged)
                        if isinstance(mc, float):
                            bias += mc
                        elif mc is not None:
                            acc = mc
                    for i, (ut, vcoef) in enumerate(mpairs):
                        vc = emit_chain(f"v{tag}{k}_{i}", vpair, vcoef)
                        if vc is None:
                            continue
                        if isinstance(vc, float):
                            if acc is None:
                                acc = pool.tile([P, J], f32, tag=f"a{tag}{k}")
                                nc.vector.tensor_scalar(
                                    acc[:, :], ut[:, :], float(vc), 0.0,
                                    OP.mult, OP.add)
                            else:
                                nc.vector.scalar_tensor_tensor(
                                    acc[:, :], ut[:, :], float(vc), acc[:, :],
                                    OP.mult, OP.add)
                        else:
                            if acc is None:
                                acc = pool.tile([P, J], f32, tag=f"a{tag}{k}")
                                nc.vector.tensor_mul(acc[:, :], ut[:, :],
                                                     vc[:, :])
                            else:
                                t = pool.tile([P, J], f32, tag=f"t{tag}{k}")
                                nc.vector.tensor_mul(t[:, :], ut[:, :],
                                                     vc[:, :])
                                nc.vector.tensor_add(acc[:, :], acc[:, :],
                                                     t[:, :])
                    outs.append((acc, bias))
                return outs

            uL = emit_side(F["W01"], F["V23"], (0, 1), (2, 3), "L")
            uR = emit_side(F["W67"], F["V45"], (6, 7), (4, 5), "R")

            # top: out = 0.5 + sum_k uL_k * uR_k  (biases folded in)
            const_out = 0.5
            acc = None
            for (aL, bL), (aR, bR) in zip(uL, uR):
                const_out += bL * bR
                for plane, b in ((aL, bR), (aR, bL)):
                    if plane is not None and abs(b) > 1e-14:
                        if acc is None:
                            acc = pool.tile([P, J], f32, tag="top")
                            nc.vector.tensor_scalar(acc[:, :], plane[:, :],
                                                    float(b), 0.0,
                                                    OP.mult, OP.add)
                        else:
                            nc.vector.scalar_tensor_tensor(
                                acc[:, :], plane[:, :], float(b), acc[:, :],
                                OP.mult, OP.add)
                if aL is not None and aR is not None:
                    if acc is None:
                        acc = pool.tile([P, J], f32, tag="top")
                        nc.vector.tensor_mul(acc[:, :], aL[:, :], aR[:, :])
                    else:
                        t = pool.tile([P, J], f32, tag="topt")
                        nc.vector.tensor_mul(t[:, :], aL[:, :], aR[:, :])
                        nc.vector.tensor_add(acc[:, :], acc[:, :], t[:, :])
            OUT = pool.tile([P, J], f32)
            if acc is None:
                nc.vector.memset(OUT[:, :], float(const_out))
            else:
                nc.vector.tensor_scalar(OUT[:, :], acc[:, :], 1.0,
                                        float(const_out), OP.mult, OP.add)

            yv = y_out.rearrange("(p j) o -> p (j o)", p=P)
            nc.sync.dma_start(yv[:, :], OUT[:, :])
    return nc


_PROGRAM_CACHE = {}
LAST_RESULT = None


def kernel(x: np.ndarray, theta: np.ndarray) -> np.ndarray:
    import os
    from concourse.bass_utils import run_bass_kernel_spmd

    x = np.ascontiguousarray(np.asarray(x, dtype=np.float32))
    theta = np.asarray(theta, dtype=np.float32)
    assert x.shape == (B_TOTAL, N_QUBITS), x.shape

    safe_range = False  # rr path measured faster than half-angle
    key = (theta.tobytes(), safe_range)
    nc = _PROGRAM_CACHE.get(key)
    if nc is None:
        F = _factorize(theta)
        nc = _build_program(F, prune_thr=_pick_prune_thr(F),
                            safe_range=safe_range)
        _PROGRAM_CACHE[key] = nc

    shards = [x[i * B_CORE:(i + 1) * B_CORE] for i in range(N_CORES)]
    in_maps = [{"x": s} for s in shards]
    trace = bool(int(os.environ.get("KERNEL_PROFILE", "0")))
    res = run_bass_kernel_spmd(nc, in_maps, list(range(N_CORES)), trace=trace)
    global LAST_RESULT
    LAST_RESULT = res
    out = np.concatenate([res.results[i]["out"] for i in range(N_CORES)], axis=0)
    return out.astype(np.float32, copy=False)



# revision 4
# speedup vs baseline: 1.3759x; 1.3759x over previous
"""Trainium2 Bass kernel for nn_BatchedQNodeLayer (8-qubit batched QNode).

Math: out_b = 0.5 + 0.5*<psi(x_b)| M(theta) |psi_b>, M expanded in the
{I,Y,Z}^8 Pauli basis factors hierarchically (operator-Schmidt rank 2 at
every cut for this shallow circuit).  With theta ~ 0.1*randn the factor
tensors are extremely sparse: pruned at 3e-3 (validated on the host
against the unpruned factors; tolerance budget is 2e-2) the whole
reduction collapses to an 18-op elementwise DAG per [128,128] fp16 plane
batch tile.

Device program (raw Bass, no TileContext; manual semaphores):
  SP   : input x via two HWDGE DMAs (j-halves), output DMA (low half)
  ACT  : sin table warm, then per half: S2=Sin(x/2), C2=Sin(pi/2-|x/2|);
         output DMA (high half)
  DVE  : |x/2| (fp32), trig finish  s=2*S2*C2, c=1-2*S2^2 (fp16 wide),
         then the chain DAG ([128,128] fp16 ops, several two-plane wide)
  POOL : 4 independent chain ops offloaded
All coefficients are baked as immediates; constants factored through the
chain so every linear combine is a single scalar_tensor_tensor.
Generic-theta fallback: the original TileContext program (fp32, range
reduction, 1e-5 pruning) is kept verbatim and selected whenever the
sparsity pattern check fails.
"""

import sys

sys.path.insert(0, "/opt/trn_rl_repo")

import numpy as np

N_QUBITS = 8
DIM = 256
N_CORES = 8
B_TOTAL = 131072
B_CORE = B_TOTAL // N_CORES  # 16384
P = 128                      # partitions
J = B_CORE // P              # 128 free elems per partition

TWO_PI = float(2.0 * np.pi)
INV_2PI = float(1.0 / (2.0 * np.pi))
MAGIC = float(1.5 * 2**23)   # fp32 round-to-nearest-integer bias
HALF_PI = float(np.pi / 2.0)

# raw monomial basis per pair: [1, sB, cB, sA, sAsB, sAcB, cA, cAsB, cAcB]
_SIGN9 = np.array([1, -1, 1, -1, 1, -1, 1, -1, 1], dtype=np.float64)


# ----------------------------------------------------------------------------
# Host-side precompute: theta -> hierarchical factor tensors
# ----------------------------------------------------------------------------

def _evolved_observable(theta):
    """M = U^dag Z0 U as dense 256x256 complex128 (numpy only)."""
    def rot(phi, th, om):
        c, s = np.cos(th / 2), np.sin(th / 2)
        return np.array([
            [np.exp(-0.5j * (phi + om)) * c, -np.exp(0.5j * (phi - om)) * s],
            [np.exp(-0.5j * (phi - om)) * s, np.exp(0.5j * (phi + om)) * c]])

    U = np.eye(DIM, dtype=np.complex128)

    def apply_1q(U, g, w):
        Ur = U.reshape([2] * N_QUBITS + [DIM])
        Ur = np.moveaxis(Ur, w, 0)
        Ur = np.tensordot(g, Ur, axes=([1], [0]))
        Ur = np.moveaxis(Ur, 0, w)
        return Ur.reshape(DIM, DIM)

    def apply_cnot(U, c, t):
        rows = np.arange(DIM)
        cbit = (rows >> (N_QUBITS - 1 - c)) & 1
        perm = np.where(cbit == 1, rows ^ (1 << (N_QUBITS - 1 - t)), rows)
        return U[perm, :]

    for l in range(2):
        for w in range(N_QUBITS):
            U = apply_1q(U, rot(*theta[l, w]), w)
        r = (l % (N_QUBITS - 1)) + 1
        for w in range(N_QUBITS):
            U = apply_cnot(U, w, (w + r) % N_QUBITS)
    z0 = 1.0 - 2.0 * ((np.arange(DIM) >> (N_QUBITS - 1)) & 1)
    return U.conj().T @ (z0[:, None] * U)


def _iyz_tensor(M):
    """Pauli coefficients over {I,Y,Z}^8 (axis order I,Y,Z per wire)."""
    I2 = np.eye(2, dtype=np.complex128)
    X = np.array([[0, 1], [1, 0]], dtype=np.complex128)
    Y = np.array([[0, -1j], [1j, 0]], dtype=np.complex128)
    Z = np.array([[1, 0], [0, -1]], dtype=np.complex128)
    T = M.reshape([2] * 16)
    perm = []
    for w in range(N_QUBITS):
        perm += [w, 8 + w]
    T = np.transpose(T, perm).reshape([4] * N_QUBITS)
    A = np.zeros((4, 4), dtype=np.complex128)
    for p, Pm in enumerate([I2, X, Y, Z]):
        A[p] = (Pm.T / 2).reshape(-1)
    for w in range(N_QUBITS):
        T = np.moveaxis(np.tensordot(A, T, axes=([1], [w])), 0, w)
    C = T.real
    idx = [0, 2, 3]
    return C[np.ix_(idx, idx, idx, idx, idx, idx, idx, idx)].copy()


def _factorize(theta, tol=1e-9):
    M = _evolved_observable(np.asarray(theta, np.float64))
    C = _iyz_tensor(M) * 0.5  # folds out = 0.5 + 0.5*ev
    S = C.reshape(81, 81)
    U, s, Vt = np.linalg.svd(S)
    K = max(1, int((s > s[0] * tol).sum()))
    A = U[:, :K] * np.sqrt(s[:K])
    Bv = Vt[:K].T * np.sqrt(s[:K])
    AL = A.reshape(9, 9, K)
    M1 = AL.reshape(9, 9 * K)
    P1, t1, Q1t = np.linalg.svd(M1, full_matrices=False)
    R1 = max(1, int((t1 > t1[0] * tol).sum()))
    W01 = P1[:, :R1] * np.sqrt(t1[:R1])                                  # [9,R1]
    V23 = Q1t[:R1].reshape(R1, 9, K) * np.sqrt(t1[:R1])[:, None, None]   # [R1,9,K]
    BR = Bv.reshape(9, 9, K).transpose(1, 0, 2)
    M2 = BR.reshape(9, 9 * K)
    P2, t2, Q2t = np.linalg.svd(M2, full_matrices=False)
    R2 = max(1, int((t2 > t2[0] * tol).sum()))
    W67 = P2[:, :R2] * np.sqrt(t2[:R2])                                  # [9,R2]
    V45 = Q2t[:R2].reshape(R2, 9, K) * np.sqrt(t2[:R2])[:, None, None]   # [R2,9,K]
    return dict(K=K, R1=R1, R2=R2, W01=W01, V23=V23, W67=W67, V45=V45)


# ----------------------------------------------------------------------------
# Full-rank reference evaluation (host), shared by both validators
# ----------------------------------------------------------------------------

def _full_eval(F, x):
    sin, cos = np.sin(x), np.cos(x)

    def feats(wA, wB):
        SA, CA = sin[:, wA], cos[:, wA]
        SB, CB = sin[:, wB], cos[:, wB]
        one = np.ones_like(SA)
        return np.stack([one, -SB, CB, -SA, SA * SB, -SA * CB,
                         CA, -CA * SB, CA * CB], 1)

    f01, f23 = feats(0, 1), feats(2, 3)
    f45, f67 = feats(4, 5), feats(6, 7)
    u01 = f01 @ F["W01"]
    v23 = np.einsum('ba,mak->bmk', f23, F["V23"])
    u67 = f67 @ F["W67"]
    v45 = np.einsum('bc,mck->bmk', f45, F["V45"])
    uLk = np.einsum('bm,bmk->bk', u01, v23)
    uRk = np.einsum('bm,bmk->bk', u67, v45)
    return (uLk * uRk).sum(1) + 0.5


def _test_inputs():
    rng = np.random.default_rng(0)
    xs = [rng.standard_normal((8192, N_QUBITS))]
    # adversarial extremes incl. the |x| ~ 5.2 tail of the real data
    grid = np.array([0.0, 0.5, -1.0, np.pi / 2, -np.pi / 2, 3.0,
                     np.pi, -np.pi, 4.7, -4.7, 5.3, -5.3])
    xs.append(grid[rng.integers(0, len(grid), (4096, N_QUBITS))])
    return np.concatenate(xs, 0)


# ----------------------------------------------------------------------------
# Specialized chain DAG: pattern check + constants + host validation
# ----------------------------------------------------------------------------

def _prune(v, thr):
    return np.where(np.abs(v) > thr, v, 0.0)


def _extract_consts(F, thr):
    """Return the specialized-DAG constants, or None if the sparsity
    pattern of the thr-pruned factors doesn't match the fast path."""
    cA = _SIGN9 * _prune(F["W01"][:, 0], thr)
    kap = _SIGN9 * _prune(F["W01"][:, 1], thr)
    vA = _SIGN9 * _prune(F["V23"][0, :, 0], thr)
    vB = _SIGN9 * _prune(F["V23"][0, :, 1], thr)
    vC = _SIGN9 * _prune(F["V23"][1, :, 0], thr)
    vD = _SIGN9 * _prune(F["V23"][1, :, 1], thr)
    wA = _SIGN9 * _prune(F["W67"][:, 0], thr)
    wB = _SIGN9 * _prune(F["W67"][:, 1], thr)
    zA = _SIGN9 * _prune(F["V45"][0, :, 0], thr)
    zB = _SIGN9 * _prune(F["V45"][0, :, 1], thr)
    zC = _SIGN9 * _prune(F["V45"][1, :, 0], thr)
    zD = _SIGN9 * _prune(F["V45"][1, :, 1], thr)

    def support_ok(v, allowed, required):
        nz = set(np.nonzero(v)[0].tolist())
        return nz <= set(allowed) and set(required) <= nz

    ok = (support_ok(cA, {5, 8}, {8})
          and support_ok(kap, {0}, {0})
          and support_ok(vA, {3, 6}, {6})
          and support_ok(vB, {4}, {4})
          and support_ok(vC, set(), set())
          and support_ok(vD, {2}, {2})
          and support_ok(wA, {6}, {6})
          and support_ok(wB, {4, 5, 7, 8}, {4, 5, 7, 8})
          and support_ok(zA, {2}, {2})
          and support_ok(zB, {4, 7}, {4, 7})
          and support_ok(zC, set(), set())
          and support_ok(zD, {6}, {6}))
    if not ok:
        return None

    SL0 = cA[8] * vA[6]
    SL1 = kap[0] * vD[2]
    SR0 = wA[6] * zA[2]
    SR1 = wB[8] * zD[6]
    if abs(SL1) < 1e-12 or abs(SR1) < 1e-12:
        return None
    C = dict(
        r1=cA[5] / cA[8],
        r2=vA[3] / vA[6],
        r3=cA[8] * vB[4] / SL1,
        r4=zB[4] / zB[7],
        r5=wB[4] / wB[5],
        r6=wB[7] / wB[8],
        r7=wB[5] / wB[8],
        r8=wA[6] * zB[7] / SR1,
        r9=(SL0 * SR0) / (SL1 * SR1),
        G2=SL1 * SR1,
    )
    if any(not np.isfinite(v) or abs(v) > 1e5 for v in C.values()):
        return None
    return {k: float(v) for k, v in C.items()}


def _dag_eval(C, x):
    """Host fp64 evaluation of the exact device DAG."""
    s, c = np.sin(x), np.cos(x)
    t1 = s[:, 0] * C["r1"] + c[:, 0]
    a01 = t1 * c[:, 1]
    b23 = s[:, 2] * C["r2"] + c[:, 2]
    p23 = s[:, 2] * s[:, 3]
    uR0 = c[:, 6] * c[:, 5]
    uL0 = a01 * b23
    e = a01 * p23
    uL1 = e * C["r3"] + c[:, 3]
    t2 = s[:, 4] * C["r4"] + c[:, 4]
    i1 = t2 * s[:, 5]
    t3 = s[:, 7] * C["r5"] + c[:, 7]
    t4 = s[:, 7] * C["r6"] + c[:, 7]
    m1 = s[:, 6] * t3
    m2 = c[:, 6] * t4
    g = m1 * C["r7"] + m2
    f1 = c[:, 6] * i1
    f2 = g * c[:, 4]
    uR1 = f1 * C["r8"] + f2
    P1 = uL0 * uR0
    P2 = uL1 * uR1
    t5 = P1 * C["r9"] + P2
    return t5 * C["G2"] + 0.5


def _pick_fast_consts(F, bound=6e-3):
    x = _test_inputs()
    full = _full_eval(F, x)
    for thr in (3e-3, 1e-3, 3e-4, 1e-4):
        C = _extract_consts(F, thr)
        if C is None:
            continue
        err = float(np.abs(_dag_eval(C, x) - full).max())
        if err < bound:
            return C, thr, err
    return None, None, None


# ----------------------------------------------------------------------------
# Fast program: raw Bass, manual semaphores
# ----------------------------------------------------------------------------

def _build_fast(C):
    from concourse import bass, mybir

    f32 = mybir.dt.float32
    f16 = mybir.dt.float16
    OP = mybir.AluOpType
    AF = mybir.ActivationFunctionType

    nc = bass.Bass()
    x_in = nc.dram_tensor("x", [B_CORE, N_QUBITS], f32, kind="ExternalInput")
    y_out = nc.dram_tensor("out", [B_CORE, 1], f32, kind="ExternalOutput")
    xv = x_in.rearrange("(p j) w -> p (j w)", p=P)      # [128, 1024] dram
    yv = y_out.rearrange("(p j) o -> p (j o)", p=P)     # [128, 128] dram

    X = nc.alloc_sbuf_tensor("X", [P, J * N_QUBITS], f32)        # (p, j*8+w)
    HA = nc.alloc_sbuf_tensor("HA", [P, N_QUBITS * J], f16)      # |x/2| w-major
    HS = nc.alloc_sbuf_tensor("HS", [P, N_QUBITS * J], f16)      # sin(x/2)
    HC = nc.alloc_sbuf_tensor("HC", [P, N_QUBITS * J], f16)      # cos(x/2)
    # TRIG: sin planes at w*J, cos planes at 1024 + w*J; padded tail so
    # two-plane strided views can be built via the rearrange trick.
    TRIG = nc.alloc_sbuf_tensor("TRIG", [P, 4 * N_QUBITS * J], f16)
    NSLOT = 20
    CH = nc.alloc_sbuf_tensor("CH", [P, NSLOT * J], f16)
    OUT = nc.alloc_sbuf_tensor("OUTP", [P, J], f32)
    hp = nc.alloc_sbuf_tensor("hp", [P, 1], f32)
    warm = nc.alloc_sbuf_tensor("warm", [P, 1], f32)

    s_in0 = nc.alloc_semaphore("s_in0")
    s_in1 = nc.alloc_semaphore("s_in1")
    s_hp = nc.alloc_semaphore("s_hp")
    s_act = nc.alloc_semaphore("s_act")
    s_trig = nc.alloc_semaphore("s_trig")
    s_pool = nc.alloc_semaphore("s_pool")
    s_dve = nc.alloc_semaphore("s_dve")
    s_out = nc.alloc_semaphore("s_out")

    # --- view helpers -------------------------------------------------------
    def wmajor_half(t, h):
        """[128, 8, 64] view of a w-major [P, 1024] tile, j-half h."""
        return t.ap()[:, 0:N_QUBITS * J].rearrange(
            "p (w j) -> p w j", j=J)[:, :, h * 64:(h + 1) * 64]

    def x_half(h):
        """[128, 8, 64] view of X's (j w) layout, iteration order (w, j)."""
        return X.ap().rearrange("p (j w) -> p w j", w=N_QUBITS)[
            :, :, h * 64:(h + 1) * 64]

    def trig_half(block, h):
        """block 0 = sin planes, 1 = cos planes."""
        base = block * N_QUBITS * J
        return TRIG.ap()[:, base:base + N_QUBITS * J].rearrange(
            "p (w j) -> p w j", j=J)[:, :, h * 64:(h + 1) * 64]

    def Sw(w):
        return TRIG.ap()[:, w * J:(w + 1) * J]

    def Cw(w):
        return TRIG.ap()[:, (N_QUBITS + w) * J:(N_QUBITS + w + 1) * J]

    def trig_pair(colA, colB):
        """[128, 2, 128] view of TRIG planes at elem cols colA < colB."""
        D = colB - colA
        assert D % J == 0 and colA + 2 * D <= 4 * N_QUBITS * J
        return TRIG.ap()[:, colA:colA + 2 * D].rearrange(
            "p (a b j) -> p a b j", a=2, j=J)[:, :, 0, :]

    def scol(w):
        return w * J

    def ccol(w):
        return (N_QUBITS + w) * J

    def slot(i):
        return CH.ap()[:, i * J:(i + 1) * J]

    def slot_pair(i):
        return CH.ap()[:, i * J:(i + 2) * J].rearrange("p (a j) -> p a j", a=2)

    # chain slot map
    B23, P23, UR0, UR1, UL0, E_UL1, T1, A01 = 0, 1, 2, 3, 4, 5, 6, 7
    T2, I1, T3, T4, M1s, M2s, G, F1, F2, PP1, PP2, T5 = (
        8, 9, 10, 11, 12, 13, 14, 15, 16, 17, 18, 19)

    # --- SP stream ----------------------------------------------------------
    nc.sync.dma_start(X.ap()[:, 0:512], xv[:, 0:512]).then_inc(s_in0, 16)
    nc.sync.dma_start(X.ap()[:, 512:1024], xv[:, 512:1024]).then_inc(s_in1, 16)
    nc.sync.wait_ge(s_dve, 1)
    nc.sync.dma_start(yv[0:64, :], OUT.ap()[0:64, :]).then_inc(s_out, 16)
    nc.sync.wait_ge(s_out, 32)

    # --- ACT stream ---------------------------------------------------------
    nc.scalar.activation(warm.ap(), warm.ap(), AF.Sin)  # preload Sin table
    nc.scalar.wait_ge(s_in0, 16)
    nc.scalar.activation(wmajor_half(HS, 0), x_half(0), AF.Sin,
                         scale=0.5).then_inc(s_act, 1)
    nc.scalar.activation(wmajor_half(HA, 0), x_half(0), AF.Abs, scale=0.5)
    nc.scalar.wait_ge(s_hp, 1)
    nc.scalar.activation(wmajor_half(HC, 0), wmajor_half(HA, 0), AF.Sin,
                         bias=hp.ap(), scale=-1.0).then_inc(s_act, 1)
    nc.scalar.wait_ge(s_in1, 16)
    nc.scalar.activation(wmajor_half(HS, 1), x_half(1), AF.Sin,
                         scale=0.5).then_inc(s_act, 1)
    nc.scalar.activation(wmajor_half(HA, 1), x_half(1), AF.Abs, scale=0.5)
    nc.scalar.activation(wmajor_half(HC, 1), wmajor_half(HA, 1), AF.Sin,
                         bias=hp.ap(), scale=-1.0).then_inc(s_act, 1)
    nc.scalar.wait_ge(s_dve, 1)
    nc.scalar.dma_start(yv[64:128, :], OUT.ap()[64:128, :]).then_inc(s_out, 16)

    # --- DVE stream ---------------------------------------------------------
    V = nc.vector
    for h in range(2):
        V.wait_ge(s_act, 2 * (h + 1))
        sh, ch = trig_half(0, h), trig_half(1, h)
        s2, c2 = wmajor_half(HS, h), wmajor_half(HC, h)
        V.scalar_tensor_tensor(sh, s2, 2.0, c2, OP.mult, OP.mult)
        V.scalar_tensor_tensor(ch, s2, -2.0, s2, OP.mult, OP.mult)
        ins = V.tensor_scalar(ch, ch, 1.0, None, OP.add)
    ins.then_inc(s_trig, 1)

    V.scalar_tensor_tensor(slot(T1), Sw(0), C["r1"], Cw(0), OP.mult, OP.add)
    V.tensor_tensor(slot(A01), slot(T1), Cw(1), OP.mult)
    V.scalar_tensor_tensor(slot(B23), Sw(2), C["r2"], Cw(2), OP.mult, OP.add)
    V.scalar_tensor_tensor(slot(T2), Sw(4), C["r4"], Cw(4), OP.mult, OP.add)
    V.tensor_tensor(slot(I1), slot(T2), Sw(5), OP.mult)
    V.scalar_tensor_tensor(slot(T3), Sw(7), C["r5"], Cw(7), OP.mult, OP.add)
    V.scalar_tensor_tensor(slot(T4), Sw(7), C["r6"], Cw(7), OP.mult, OP.add)
    V.wait_ge(s_pool, 1)
    # (uL0, e) = a01 * (b23, p23)
    a01b = slot(A01).rearrange("p (a j) -> p a j", a=1).broadcast_to([P, 2, J])
    V.tensor_tensor(slot_pair(UL0), a01b, slot_pair(B23), OP.mult)
    V.scalar_tensor_tensor(slot(E_UL1), slot(E_UL1), C["r3"], Cw(3),
                           OP.mult, OP.add)
    # (m1, m2) = (s6, c6) * (t3, t4)
    V.tensor_tensor(slot_pair(M1s),
                    trig_pair(scol(6), ccol(6)),
                    slot_pair(T3), OP.mult)
    V.scalar_tensor_tensor(slot(G), slot(M1s), C["r7"], slot(M2s),
                           OP.mult, OP.add)
    V.tensor_tensor(slot(F1), Cw(6), slot(I1), OP.mult)
    V.tensor_tensor(slot(F2), slot(G), Cw(4), OP.mult)
    V.scalar_tensor_tensor(slot(UR1), slot(F1), C["r8"], slot(F2),
                           OP.mult, OP.add)
    # (P1, P2) = (uL0, uL1) * (uR0, uR1)
    V.tensor_tensor(slot_pair(PP1), slot_pair(UL0), slot_pair(UR0), OP.mult)
    V.scalar_tensor_tensor(slot(T5), slot(PP1), C["r9"], slot(PP2),
                           OP.mult, OP.add)
    V.tensor_scalar(OUT.ap(), slot(T5), C["G2"], 0.5,
                    OP.mult, OP.add).then_inc(s_dve, 1)

    # --- POOL stream --------------------------------------------------------
    # Pool's ISA has no TENSOR_SCALAR_PTR; it only gets pure tensor_tensor
    # work: the (p23, uR0) = (s2*s3, c6*c5) two-plane product.
    G_ = nc.gpsimd
    G_.memset(hp.ap(), HALF_PI).then_inc(s_hp, 1)
    G_.wait_ge(s_trig, 1)
    G_.tensor_tensor(slot_pair(P23),
                     trig_pair(scol(2), ccol(6)),
                     trig_pair(scol(3), ccol(5)), OP.mult).then_inc(s_pool, 1)

    return nc


# ----------------------------------------------------------------------------
# Fallback program: original TileContext build (any theta), fp32
# ----------------------------------------------------------------------------

def _prune_err(F, thr):
    x = _test_inputs()
    full = _full_eval(F, x)
    Fp = dict(F)
    for k in ("W01", "V23", "W67", "V45"):
        Fp[k] = _prune(F[k], thr)
    return float(np.abs(_full_eval(Fp, x) - full).max())


def _pick_prune_thr(F):
    for thr in (1e-5, 1e-6, 1e-7, 0.0):
        if _prune_err(F, thr) < 3e-5:
            return thr
    return 0.0


def _build_program(F, prune_thr=1e-5, safe_range=True):
    from concourse import bass, mybir, tile

    class SafeTileContext(tile.TileContext):
        """Reject instructions carrying more than one sync wait; park every
        extra wait on a same-engine nop inserted immediately before."""

        def schedule_and_allocate(self):
            ret = super().schedule_and_allocate()
            nc = self.nc
            for bb in list(nc.main_func.blocks):
                i = 0
                while i < len(bb.instructions):
                    ins = bb.instructions[i]
                    si = ins.sync_info
                    waits = list(si.on_wait or []) if si else []
                    lim = 1
                    if len(waits) > lim:
                        ins.sync_info = mybir.SyncInfo(
                            on_wait=waits[:lim], on_update=si.on_update)
                        rest = waits[lim:]
                        nops = []
                        while rest:
                            n = nc.engines[ins.engine].nop()
                            n.ins.sync_info = mybir.SyncInfo(
                                on_wait=rest[:1], on_update=[])
                            rest = rest[1:]
                            nops.append(n.ins)
                        for n in nops:
                            for blk in nc.main_func.blocks:
                                if n in blk.instructions:
                                    blk.instructions.remove(n)
                                    break
                        bb.instructions[i:i] = nops
                        i += len(nops)
                    i += 1
            return ret

    f32 = mybir.dt.float32
    OP = mybir.AluOpType
    AF = mybir.ActivationFunctionType

    nc = bass.Bass()
    x_in = nc.dram_tensor("x", [B_CORE, N_QUBITS], f32, kind="ExternalInput")
    y_out = nc.dram_tensor("out", [B_CORE, 1], f32, kind="ExternalOutput")

    with SafeTileContext(nc) as tc:
        with tc.tile_pool(name="pool", bufs=1) as pool:
            X = pool.tile([P, J * N_QUBITS], f32)        # (p, j*8+w)
            T1 = pool.tile([P, J * N_QUBITS], f32)
            Y = pool.tile([P, N_QUBITS * J], f32)        # w-major (p, w*128+j)
            TRIG = pool.tile([P, 2 * N_QUBITS * J], f32)
            AB = pool.tile([P, N_QUBITS * J], f32)
            PROD = pool.tile([P, 16 * J], f32)
            hp = pool.tile([P, 1], f32)

            warm = pool.tile([P, 1], f32)
            nc.scalar.activation(warm[:, :], warm[:, :], AF.Sin)

            nc.vector.memset(hp[:, :], HALF_PI)

            xv = x_in.rearrange("(p j) w -> p (j w)", p=P)
            nc.gpsimd.dma_start(X[0:64, :], xv[0:64, :])
            nc.gpsimd.dma_start(X[64:128, :], xv[64:128, :])

            SIN = TRIG[:, 0:N_QUBITS * J]
            COS = TRIG[:, N_QUBITS * J:2 * N_QUBITS * J]
            H = 4 * J
            # range reduction: y = x - 2pi*round(x/(2pi)), w-major
            nc.vector.tensor_scalar(T1[:, :], X[:, :], INV_2PI, MAGIC,
                                    OP.mult, OP.add)
            nc.vector.tensor_scalar(T1[:, :], T1[:, :], MAGIC, None,
                                    OP.subtract)
            for w in range(N_QUBITS):
                Yw = Y[:, w * J:(w + 1) * J]
                T1w = T1[:, :].rearrange("p (j w) -> p w j",
                                         w=N_QUBITS)[:, w, :]
                Xw = X[:, :].rearrange("p (j w) -> p w j",
                                       w=N_QUBITS)[:, w, :]
                nc.vector.scalar_tensor_tensor(Yw, T1w, -TWO_PI, Xw,
                                               OP.mult, OP.add)
            for h in range(2):
                sl = slice(h * H, (h + 1) * H)
                nc.scalar.activation(SIN[:, sl], Y[:, sl], AF.Sin)
                nc.scalar.activation(AB[:, sl], Y[:, sl], AF.Abs)
                nc.scalar.activation(COS[:, sl], AB[:, sl], AF.Sin,
                                     bias=hp[:, :], scale=-1.0)

            def Sw(w):
                return TRIG[:, w * J:(w + 1) * J]

            def Cw(w):
                return TRIG[:, (N_QUBITS + w) * J:(N_QUBITS + w + 1) * J]

            tv = TRIG[:, :].rearrange("p (a pr t j) -> p a pr t j",
                                      a=2, pr=4, t=2)
            ov = PROD[:, :].rearrange("p (pr a b j) -> p pr a b j",
                                      pr=4, a=2, b=2)
            in2 = tv[:, :, :, 1:2, :].transpose([0, 2, 1, 3, 4]) \
                .squeeze(3)
            for h in range(2):
                pr = slice(2 * h, 2 * h + 2)
                for a in range(2):
                    in1 = tv[:, a:a + 1, pr, 0:1, :].squeeze(1) \
                        .broadcast_to([P, 2, 2, J])
                    out_a = ov[:, pr, a:a + 1, :, :].squeeze(2)
                    nc.vector.tensor_tensor(out_a, in1[:, :, :, :],
                                            in2[:, pr, :, :], OP.mult)

            def prod(pair_idx, a, b):
                base = (pair_idx * 4 + a * 2 + b) * J
                return PROD[:, base:base + J]

            PAIR_IDX = {(0, 1): 0, (2, 3): 1, (4, 5): 2, (6, 7): 3}
            PRUNE = float(prune_thr)

            def emit_chain(name, pair, w9):
                wA, wB = pair
                pi = PAIR_IDX[pair]
                cand = [
                    (Sw(wB), -w9[1]), (Cw(wB), w9[2]),
                    (Sw(wA), -w9[3]), (Cw(wA), w9[6]),
                    (prod(pi, 0, 0), w9[4]), (prod(pi, 0, 1), -w9[5]),
                    (prod(pi, 1, 0), -w9[7]), (prod(pi, 1, 1), w9[8]),
                ]
                terms = [(ap, c) for (ap, c) in cand if abs(c) > PRUNE]
                if not terms:
                    if abs(w9[0]) <= PRUNE:
                        return None
                    return float(w9[0])
                q = pool.tile([P, J], f32, tag=name)
                ap0, c0 = terms[0]
                nc.vector.tensor_scalar(q[:, :], ap0, float(c0), float(w9[0]),
                                        OP.mult, OP.add)
                for (ap, c) in terms[1:]:
                    nc.vector.scalar_tensor_tensor(q[:, :], ap, float(c),
                                                   q[:, :], OP.mult, OP.add)
                return q

            def emit_side(Wu, Vv, upair, vpair, tag):
                R = Wu.shape[1]
                K = Vv.shape[2]
                us = [emit_chain(f"u{tag}{m}", upair, Wu[:, m])
                      for m in range(R)]
                outs = []
                for k in range(K):
                    merged = np.zeros(9)
                    mpairs = []
                    for m in range(R):
                        vcoef = Vv[m, :, k]
                        if not np.any(np.abs(vcoef) > PRUNE):
                            continue
                        if us[m] is None:
                            continue
                        if isinstance(us[m], float):
                            merged = merged + us[m] * vcoef
                        else:
                            mpairs.append((us[m], vcoef))
                    acc = None
                    bias = 0.0
                    if np.any(np.abs(merged) > PRUNE):
                        mc = emit_chain(f"w{tag}{k}", vpair, merged)
                        if isinstance(mc, float):
                            bias += mc
                        elif mc is not None:
                            acc = mc
                    for i, (ut, vcoef) in enumerate(mpairs):
                        vc = emit_chain(f"v{tag}{k}_{i}", vpair, vcoef)
                        if vc is None:
                            continue
                        if isinstance(vc, float):
                            if acc is None:
                                acc = pool.tile([P, J], f32, tag=f"a{tag}{k}")
                                nc.vector.tensor_scalar(
                                    acc[:, :], ut[:, :], float(vc), 0.0,
                                    OP.mult, OP.add)
                            else:
                                nc.vector.scalar_tensor_tensor(
                                    acc[:, :], ut[:, :], float(vc), acc[:, :],
                                    OP.mult, OP.add)
                        else:
                            if acc is None:
                                acc = pool.tile([P, J], f32, tag=f"a{tag}{k}")
                                nc.vector.tensor_mul(acc[:, :], ut[:, :],
                                                     vc[:, :])
                            else:
                                t = pool.tile([P, J], f32, tag=f"t{tag}{k}")
                                nc.vector.tensor_mul(t[:, :], ut[:, :],
                                                     vc[:, :])
                                nc.vector.tensor_add(acc[:, :], acc[:, :],
                                                     t[:, :])
                    outs.append((acc, bias))
                return outs

            uL = emit_side(F["W01"], F["V23"], (0, 1), (2, 3), "L")
            uR = emit_side(F["W67"], F["V45"], (6, 7), (4, 5), "R")

            const_out = 0.5
            acc = None
            for (aL, bL), (aR, bR) in zip(uL, uR):
                const_out += bL * bR
                for plane, b in ((aL, bR), (aR, bL)):
                    if plane is not None and abs(b) > 1e-14:
                        if acc is None:
                            acc = pool.tile([P, J], f32, tag="top")
                            nc.vector.tensor_scalar(acc[:, :], plane[:, :],
                                                    float(b), 0.0,
                                                    OP.mult, OP.add)
                        else:
                            nc.vector.scalar_tensor_tensor(
                                acc[:, :], plane[:, :], float(b), acc[:, :],
                                OP.mult, OP.add)
                if aL is not None and aR is not None:
                    if acc is None:
                        acc = pool.tile([P, J], f32, tag="top")
                        nc.vector.tensor_mul(acc[:, :], aL[:, :], aR[:, :])
                    else:
                        t = pool.tile([P, J], f32, tag="topt")
                        nc.vector.tensor_mul(t[:, :], aL[:, :], aR[:, :])
                        nc.vector.tensor_add(acc[:, :], acc[:, :], t[:, :])
            OUT = pool.tile([P, J], f32)
            if acc is None:
                nc.vector.memset(OUT[:, :], float(const_out))
            else:
                nc.vector.tensor_scalar(OUT[:, :], acc[:, :], 1.0,
                                        float(const_out), OP.mult, OP.add)

            yv = y_out.rearrange("(p j) o -> p (j o)", p=P)
            nc.sync.dma_start(yv[:, :], OUT[:, :])
    return nc


_PROGRAM_CACHE = {}
LAST_RESULT = None
LAST_PATH = None


def kernel(x: np.ndarray, theta: np.ndarray) -> np.ndarray:
    import os
    from concourse.bass_utils import run_bass_kernel_spmd

    x = np.ascontiguousarray(np.asarray(x, dtype=np.float32))
    theta = np.asarray(theta, dtype=np.float32)
    assert x.shape == (B_TOTAL, N_QUBITS), x.shape

    global LAST_PATH
    key = theta.tobytes()
    cached = _PROGRAM_CACHE.get(key)
    if cached is None:
        F = _factorize(theta)
        C, thr, err = _pick_fast_consts(F)
        if C is not None:
            nc = _build_fast(C)
            LAST_PATH = f"fast(thr={thr:g}, host_err={err:.2e})"
        else:
            nc = _build_program(F, prune_thr=_pick_prune_thr(F),
                                safe_range=False)
            LAST_PATH = "fallback"
        _PROGRAM_CACHE[key] = (nc, LAST_PATH)
    else:
        nc, LAST_PATH = cached

    shards = [x[i * B_CORE:(i + 1) * B_CORE] for i in range(N_CORES)]
    in_maps = [{"x": s} for s in shards]
    trace = bool(int(os.environ.get("KERNEL_PROFILE", "0")))
    res = run_bass_kernel_spmd(nc, in_maps, list(range(N_CORES)), trace=trace)
    global LAST_RESULT
    LAST_RESULT = res
    out = np.concatenate([res.results[i]["out"] for i in range(N_CORES)],
                         axis=0)
    return out.astype(np.float32, copy=False)
